# revision 21
# baseline (speedup 1.0000x reference)
"""Trainium2 Bass kernel for nn_DecoderBlock (S=4096, D=768, H=12).

Strategy (8 NeuronCores, SPMD):
  - Sequence-parallel: core c owns rows [c*512, (c+1)*512) of the sequence.
  - All activations kept in transposed layout (feature dim on partitions,
    sequence on the free axis).  LayerNorm statistics are computed with
    ones-vector matmuls (partition-axis reduction on the PE).
  - K (transposed) and natural-layout V are exchanged with G=6 chunked
    AllGather collectives (bf16), one head-PAIR per chunk, so attention on
    head pair hp starts as soon as chunk hp lands; kT / v SBUF buffers
    rotate (bufs=2) instead of staying fully resident.
  - Attention per head pair hp, per sk-tile b: two score matmuls (K=64)
    into one [128,1024] PSUM tile, one Exp covers both heads, then the PV
    runs TRANSPOSED: the exp tile is the STATIONARY operand ([128 sk,
    128 q] blocks) and V rides moving with an appended ones column
    ([128 sk, 65]) so each PV matmul emits context^T[q, dh] AND the
    softmax denominator (col 64) for free — no separate denominator
    matmuls.  The normalize is then a per-partition tensor_scalar on DVE
    (reciprocal via reciprocal_approx_fast, also DVE, keeping the Scalar
    engine free for the Exp stream), and a PE transpose (vs a resident
    identity) brings ctx back to feature-major for the residual.
"""

import os
import sys

for _p in ("/opt/trn_rl_repo", os.path.expanduser("~/.axon_site/_ro/trn_rl_repo")):
    if os.path.isdir(_p) and _p not in sys.path:
        sys.path.append(_p)

import numpy as np
from contextlib import ExitStack

import concourse.bass as bass
import concourse.tile as tile
from concourse import bacc, mybir, masks

F32 = mybir.dt.float32
BF16 = mybir.dt.bfloat16
F8 = mybir.dt.float8e4
AF = mybir.ActivationFunctionType
ALU = mybir.AluOpType


class Cfg:
    def __init__(self, S=4096, D=768, H=12, NC=8, eps=1e-5):
        self.S, self.D, self.H, self.NC, self.eps = S, D, H, NC, eps
        self.DH = D // H
        assert self.DH == 64 and H % 2 == 0 and D % 128 == 0
        self.SL = S // NC          # local sequence rows per core
        assert self.SL % 128 == 0 and self.SL <= 512
        self.ND = D // 128         # d tiles
        self.HP = H // 2           # head pairs
        self.G = self.HP           # one allgather chunk per head pair
        self.NSK = S // 128        # sk tiles (global)
        self.NFF = 4 * D // 128    # ffn hidden tiles
        self.TS = self.SL // 128   # local sk tiles


def build(cfg: Cfg, debug=False, enable_asserts=False):
    nc = bacc.Bacc(
        "TRN2",
        target_bir_lowering=False,
        debug=debug,
        enable_asserts=enable_asserts,
        num_devices=cfg.NC,
    )
    S, D, H, SL, G = cfg.S, cfg.D, cfg.H, cfg.SL, cfg.G
    ND, HP, NSK, NFF, NC, TS = cfg.ND, cfg.HP, cfg.NSK, cfg.NFF, cfg.NC, cfg.TS
    scale = 1.0 / float(np.sqrt(cfg.DH))

    # ---- DRAM I/O ----------------------------------------------------------
    xT = nc.dram_tensor("xT", [D, SL], F32, kind="ExternalInput").ap()
    w_qkT = nc.dram_tensor("w_qkT", [D, 2 * D], BF16, kind="ExternalInput").ap()
    w_vT = nc.dram_tensor("w_vT", [D, D], BF16, kind="ExternalInput").ap()
    b_qk = nc.dram_tensor("b_qk", [128, 2 * D // 128], F32, kind="ExternalInput").ap()
    b_v = nc.dram_tensor("b_v", [1, D], F32, kind="ExternalInput").ap()
    ln1w = nc.dram_tensor("ln1w", [128, ND], F32, kind="ExternalInput").ap()
    ln1b = nc.dram_tensor("ln1b", [128, ND], F32, kind="ExternalInput").ap()
    ln2w = nc.dram_tensor("ln2w", [128, ND], F32, kind="ExternalInput").ap()
    ln2b = nc.dram_tensor("ln2b", [128, ND], F32, kind="ExternalInput").ap()
    w_fcT = nc.dram_tensor("w_fcT", [D, 4 * D], BF16, kind="ExternalInput").ap()
    b_fc = nc.dram_tensor("b_fc", [128, NFF], F32, kind="ExternalInput").ap()
    w_projT = nc.dram_tensor("w_projT", [4 * D, D], BF16, kind="ExternalInput").ap()
    b_proj = nc.dram_tensor("b_proj", [128, ND], F32, kind="ExternalInput").ap()
    outT = nc.dram_tensor("outT", [D, SL], F32, kind="ExternalOutput").ap()

    with tile.TileContext(nc) as tc, ExitStack() as top:
        persist = top.enter_context(tc.tile_pool(name="persist", bufs=1))
        dram = top.enter_context(tc.tile_pool(name="dram", bufs=1, space="DRAM"))

        # tiny warmup collective fired as the very first thing: absorbs the
        # first-collective setup/rendezvous cost while LN1/QKV compute runs
        gspace0 = "Shared" if cfg.NC > 4 else "Local"
        warm_own = dram.tile([128], BF16, name="warm_own")
        warm_gath = dram.tile([cfg.NC * 128], BF16, addr_space=gspace0,
                              name="warm_gath")
        warm_sb = persist.tile([1, 128], BF16)
        nc.vector.memset(warm_sb[:], 0.0)
        wview = warm_own[:].rearrange("(o s) -> o s", o=1)
        nc.sync.dma_start(wview[:], warm_sb[:])
        nc.gpsimd.collective_compute(
            "AllGather", ALU.bypass, replica_groups=[list(range(cfg.NC))],
            ins=[warm_own[:]], outs=[warm_gath[:]])

        # constants / parameters resident in SBUF
        ones_col = persist.tile([128, 1], F32)
        nc.vector.memset(ones_col[:], 1.0)
        ones33 = persist.tile([33, 128], F32)
        nc.vector.memset(ones33[:], 1.0)
        eps_tile = persist.tile([1, 1], F32)
        nc.vector.memset(eps_tile[:], float(cfg.eps))
        ident = persist.tile([128, 128], BF16)
        masks.make_identity(nc, ident[:])

        b_qk_sb = persist.tile([128, 2 * D // 128], F32)
        nc.sync.dma_start(b_qk_sb[:], b_qk[:])
        ln1w_sb = persist.tile([128, ND], F32)
        nc.sync.dma_start(ln1w_sb[:], ln1w[:])
        ln1b_sb = persist.tile([128, ND], F32)
        nc.sync.dma_start(ln1b_sb[:], ln1b[:])
        ln2w_sb = persist.tile([128, ND], F32)
        nc.sync.dma_start(ln2w_sb[:], ln2w[:])
        ln2b_sb = persist.tile([128, ND], F32)
        nc.sync.dma_start(ln2b_sb[:], ln2b[:])
        b_fc_sb = persist.tile([128, NFF], F32)
        nc.sync.dma_start(b_fc_sb[:], b_fc[:])
        b_proj_sb = persist.tile([128, ND], F32)
        nc.sync.dma_start(b_proj_sb[:], b_proj[:])
        b_v_sb = persist.tile([1, D], F32)
        nc.sync.dma_start(b_v_sb[:], b_v[:])

        # persistent activations
        ln1x = [persist.tile([128, SL], F32, name=f"ln1x{t}") for t in range(ND)]
        ctxu = [persist.tile([128, SL], F32, name=f"ctxu{t}") for t in range(ND)]
        q_sb = [persist.tile([128, SL], BF16, name=f"q_sb{t}") for t in range(HP)]

        def layernorm_T(src_tiles, w_sb, b_sb, out_f32, out_bf16):
            """LayerNorm over the partition (feature) axis of transposed tiles."""
            with tc.tile_pool(name="ln_ps", bufs=1, space="PSUM") as lps, \
                 tc.tile_pool(name="ln_sb", bufs=2) as lsb:
                sums = lps.tile([1, SL], F32, tag="st", bufs=2)
                sumsq = lps.tile([1, SL], F32, tag="st", bufs=2)
                sq = [lsb.tile([128, SL], F32, tag="lntmp", bufs=2, name=f"sq{t}")
                      for t in range(ND)]
                for t in range(ND):
                    nc.vector.tensor_tensor(sq[t][:], src_tiles[t][:],
                                            src_tiles[t][:], op=ALU.mult)
                for t in range(ND):
                    nc.tensor.matmul(sums[:], ones_col[:], src_tiles[t][:],
                                     start=(t == 0), stop=(t == ND - 1))
                for t in range(ND):
                    nc.tensor.matmul(sumsq[:], ones_col[:], sq[t][:],
                                     start=(t == 0), stop=(t == ND - 1))
                mean = lsb.tile([1, SL], F32)
                ex2 = lsb.tile([1, SL], F32)
                msq = lsb.tile([1, SL], F32)
                var = lsb.tile([1, SL], F32)
                lnv = lsb.tile([1, SL], F32)
                rstd = lsb.tile([1, SL], F32)
                nc.vector.tensor_scalar_mul(mean[:], sums[:], 1.0 / D)
                nc.vector.tensor_scalar_mul(ex2[:], sumsq[:], 1.0 / D)
                nc.vector.tensor_tensor(msq[:], mean[:], mean[:], op=ALU.mult)
                nc.vector.tensor_tensor(var[:], ex2[:], msq[:], op=ALU.subtract)
                nc.scalar.activation(lnv[:], var[:], AF.Ln, bias=eps_tile[:])
                nc.scalar.activation(rstd[:], lnv[:], AF.Exp, scale=-0.5)
                with tc.tile_pool(name="lnb_ps", bufs=1, space="PSUM") as bps:
                    meanB = bps.tile([128, SL], F32, tag="bc", bufs=2)
                    rstdB = bps.tile([128, SL], F32, tag="bc", bufs=2)
                    nc.tensor.matmul(meanB[:], ones33[0:1, :], mean[:],
                                     start=True, stop=True)
                    nc.tensor.matmul(rstdB[:], ones33[0:1, :], rstd[:],
                                     start=True, stop=True)
                    for t in range(ND):
                        cen = lsb.tile([128, SL], F32, tag="lntmp", bufs=2,
                                       name=f"cen{t}")
                        nc.vector.tensor_tensor(cen[:], src_tiles[t][:],
                                                meanB[:], op=ALU.subtract)
                        nc.vector.tensor_tensor(cen[:], cen[:], rstdB[:],
                                                op=ALU.mult)
                        # DVE (not ACT Identity): avoids activation-table
                        # reloads around the Exp/Gelu streams
                        nc.vector.tensor_scalar(out_f32[t][:], cen[:],
                                                w_sb[:, t:t + 1],
                                                b_sb[:, t:t + 1],
                                                op0=ALU.mult, op1=ALU.add)
                        nc.scalar.copy(out_bf16[t][:], out_f32[t][:])

        # ==== phase 1: LN1 ====================================================
        p12 = tc.alloc_tile_pool(name="p12", bufs=1)
        ln1xb = [p12.tile([128, SL], BF16, name=f"ln1xb{t}") for t in range(ND)]
        with tc.tile_pool(name="xin", bufs=1) as xin:
            x_sb = [xin.tile([128, SL], F32, name=f"x_sb{t}") for t in range(ND)]
            for t in range(ND):
                nc.sync.dma_start(x_sb[t][:], xT[128 * t:128 * (t + 1), :])
            layernorm_T(x_sb, ln1w_sb, ln1b_sb, ln1x, ln1xb)

        # ==== phase 2: qkv + v, write own k/v to DRAM (one chunk per hp) ======
        # chunk g covers feature rows [g*128, (g+1)*128) of k^T / columns of v;
        # each chunk's k and v ride in ONE flat allgather (k block then v block)
        CB = 128 * SL        # elements of the k block per chunk
        VB = SL * 130        # elements of the ones-augmented v block
        BLK = CB + VB
        kv_own = [dram.tile([BLK], F8, name=f"kv_own{g}") for g in range(G)]
        gspace = "Shared" if NC > 4 else "Local"
        kv_gath = [dram.tile([NC * BLK], F8, addr_space=gspace,
                             name=f"kv_gath{g}") for g in range(G)]
        grp = [list(range(NC))]



        with tc.tile_pool(name="wqkv", bufs=1) as wp, \
             tc.tile_pool(name="qkv_ps", bufs=1, space="PSUM") as qps, \
             tc.tile_pool(name="kv_sb", bufs=1) as kvp:
            w_qk_sb = [wp.tile([128, 2 * D], BF16, name=f"wqk{t}") for t in range(ND)]
            w_v_sb = [wp.tile([128, D], BF16, name=f"wv{t}") for t in range(ND)]
            for t in range(ND):
                nc.sync.dma_start(w_qk_sb[t][:], w_qkT[128 * t:128 * (t + 1), :])
                nc.sync.dma_start(w_v_sb[t][:], w_vT[128 * t:128 * (t + 1), :])

            bvb_sb = kvp.tile([128, D], F32)
            with tc.tile_pool(name="bv_ps", bufs=1, space="PSUM") as bvp:
                BC = D // ((D + 511) // 512)
                for i in range(D // BC):
                    bvb = bvp.tile([128, 512], F32, tag="bvb", bufs=2,
                                   name=f"bvb{i}")
                    nc.tensor.matmul(bvb[:, 0:BC], ones33[0:1, :],
                                     b_v_sb[:, BC * i:BC * (i + 1)],
                                     start=True, stop=True)
                    nc.vector.tensor_copy(bvb_sb[:, BC * i:BC * (i + 1)],
                                          bvb[:, 0:BC])
            # ones columns for the v̂ DRAM blocks, written via DMA per chunk
            ones_v = kvp.tile([128, TS], F8)
            nc.vector.memset(ones_v[:], 1.0)
            onesrc = ones_v.rearrange("p (t z) -> p t z", z=1)

            def emit_chunk(g, vps):
                """bias-add chunk g's v slice from group PSUM, write k+v̂ to
                kv_own[g], and launch its allgather."""
                j = ND + g
                ps = qps.tile([128, SL], F32, tag="qk", bufs=2)
                for t in range(ND):
                    nc.tensor.matmul(ps[:],
                                     w_qk_sb[t][:, 128 * j:128 * (j + 1)],
                                     ln1xb[t][:], start=(t == 0),
                                     stop=(t == ND - 1))
                k_sb = kvp.tile([128, SL], F8, tag="k_sb", bufs=2,
                                name=f"k_sb{g}")
                nc.vector.tensor_scalar(k_sb[:], ps[:],
                                        b_qk_sb[:, j:j + 1], None,
                                        op0=ALU.add)
                kview = kv_own[g][0:CB].rearrange("(d s) -> d s", s=SL)
                nc.sync.dma_start(kview[:], k_sb[:])
                lo, off = 128 * g, 128 * (g % 3)
                stage = kvp.tile([128, 512], F8, tag="vstg", bufs=2,
                                 name=f"vstg{g}")
                for m in range(TS):
                    nc.vector.tensor_tensor(stage[:, 128 * m:128 * (m + 1)],
                                            vps[m][:, off:off + 128],
                                            bvb_sb[:, lo:lo + 128], op=ALU.add)
                vview = kv_own[g][CB:BLK].rearrange("(m p z) -> p m z",
                                                    p=128, z=130)
                ssrc = stage.rearrange("p (m w) -> p m w", w=128)
                nc.sync.dma_start(vview[:, :, 0:64], ssrc[:, :, 0:64])
                nc.sync.dma_start(vview[:, :, 65:129], ssrc[:, :, 64:128])
                nc.sync.dma_start(vview[:, :, 64:65], onesrc[:])
                nc.sync.dma_start(vview[:, :, 129:130], onesrc[:])
                nc.gpsimd.collective_compute(
                    "AllGather", ALU.bypass, replica_groups=grp,
                    ins=[kv_own[g][:]], outs=[kv_gath[g][:]])

            # v for 3 chunks at a time with N=384 matmuls; k per chunk is
            # N=512.  Chunk 0's allgather is in flight ~15us in.
            for grp3 in range(2):
                vps = [qps.tile([128, 384], F32, tag="v", bufs=4,
                                name=f"vps{grp3}_{m}") for m in range(TS)]
                for m in range(TS):
                    for t in range(ND):
                        nc.tensor.matmul(
                            vps[m][:], ln1xb[t][:, 128 * m:128 * (m + 1)],
                            w_v_sb[t][:, 384 * grp3:384 * (grp3 + 1)],
                            start=(t == 0), stop=(t == ND - 1))
                for g in range(3 * grp3, 3 * grp3 + 3):
                    emit_chunk(g, vps)

            # q projections last (only needed once attention starts)
            for j in range(ND):
                ps = qps.tile([128, SL], F32, tag="qk", bufs=2)
                for t in range(ND):
                    nc.tensor.matmul(ps[:], w_qk_sb[t][:, 128 * j:128 * (j + 1)],
                                     ln1xb[t][:], start=(t == 0), stop=(t == ND - 1))
                nc.vector.tensor_scalar(q_sb[j][:], ps[:], b_qk_sb[:, j:j + 1],
                                        None, op0=ALU.add)
        p12.release()

        # ==== phase 4: attention =============================================
        # v̂ per sk-tile block: [A(64) | onesA | B(64) | onesB] (width 130)
        with tc.tile_pool(name="attn_sb", bufs=1) as ap, \
             tc.tile_pool(name="sg_ps", bufs=1, space="PSUM") as sps, \
             tc.tile_pool(name="cs_ps", bufs=1, space="PSUM") as cps, \
             tc.tile_pool(name="exp_sb", bufs=1) as epool:
            for hp in range(HP):
                kT8 = ap.tile([128, S], F8, tag="kT8", bufs=2, name=f"kT8_{hp}")
                vh8 = ap.tile([128, NSK * 130], F8, tag="vh8", bufs=2,
                              name=f"vh8_{hp}")
                gat = kv_gath[hp].rearrange("(c b) -> c b", b=BLK)
                ksrc = gat[:, 0:CB].rearrange("c (d s) -> d c s", s=SL)
                kdst = kT8.rearrange("p (c s) -> p c s", c=NC)
                nc.sync.dma_start(kdst[:], ksrc[:])
                vsrc = gat[:, CB:BLK].rearrange("c (t p z) -> p c t z",
                                                p=128, z=130)
                vdst = vh8.rearrange("p (c t z) -> p c t z", c=NC, t=TS, z=130)
                for c in range(NC):
                    nc.sync.dma_start(vdst[:, c], vsrc[:, c])
                kT = ap.tile([128, S], BF16, tag="kT", bufs=2, name=f"kT{hp}")
                vh = ap.tile([128, NSK * 130], BF16, tag="vh", bufs=2,
                             name=f"vh{hp}")
                # chunked upcasts: the first score/PV matmuls only depend on
                # the first quarter, not the whole 4K-wide copy
                for u in range(4):
                    nc.vector.tensor_copy(kT[:, 1024 * u:1024 * (u + 1)],
                                          kT8[:, 1024 * u:1024 * (u + 1)])
                    nc.vector.tensor_copy(vh[:, 1040 * u:1040 * (u + 1)],
                                          vh8[:, 1040 * u:1040 * (u + 1)])

                # ctxT accumulators: 2 banks x 4 accumulators of [128q, 65]
                # bank h holds head h's 4 q-blocks
                ctxT = [cps.tile([128, 4 * 65], F32, tag=f"ctxT{h}", bufs=1,
                                 name=f"ctxT{h}_{hp}") for h in range(2)]
                for b in range(NSK):
                    sg = sps.tile([128, 2 * SL], F32, tag="sg", bufs=2)
                    nc.tensor.matmul(sg[:, 0:SL],
                                     kT[0:64, 128 * b:128 * (b + 1)],
                                     q_sb[hp][0:64, :], start=True, stop=True)
                    nc.tensor.matmul(sg[:, SL:2 * SL],
                                     kT[64:128, 128 * b:128 * (b + 1)],
                                     q_sb[hp][64:128, :], start=True, stop=True)
                    ex = epool.tile([128, 2 * SL], BF16, tag="exp", bufs=3)
                    nc.scalar.activation(ex[:], sg[:], AF.Exp, scale=scale)
                    st, sp = (b == 0), (b == NSK - 1)
                    for h in range(2):
                        for qb in range(4):
                            nc.tensor.matmul(
                                ctxT[h][:, 65 * qb:65 * (qb + 1)],
                                ex[:, SL * h + 128 * qb:SL * h + 128 * (qb + 1)],
                                vh[:, 130 * b + 65 * h:130 * b + 65 * (h + 1)],
                                start=st, stop=sp, skip_group_check=True)
                # epilogue: reciprocal of denominators (DVE), per-partition
                # normalize (DVE), PE transpose back to feature-major,
                # residual add -> ctxu[hp]
                rcp = epool.tile([128, 8], F32, tag="rcp", bufs=2)
                for h in range(2):
                    dv = ctxT[h].rearrange("p (a z) -> p a z", z=65)
                    nc.vector.reciprocal_approx_fast(
                        rcp.rearrange("p (a z) -> p a z", z=1)[:, 4 * h:4 * h + 4, :],
                        dv[:, :, 64:65])
                for qb in range(4):
                    cn = epool.tile([128, 128], BF16, tag="cn", bufs=3,
                                    name=f"cn{hp}_{qb}")
                    for h in range(2):
                        nc.vector.tensor_scalar(
                            cn[:, 64 * h:64 * (h + 1)],
                            ctxT[h][:, 65 * qb:65 * qb + 64],
                            rcp[:, 4 * h + qb:4 * h + qb + 1], None,
                            op0=ALU.mult)
                    tp = cps.tile([128, 128], BF16, tag="tp", bufs=2,
                                  name=f"tp{hp}_{qb}")
                    nc.tensor.matmul(tp[:], cn[:], ident[:], is_transpose=True,
                                     skip_group_check=True)
                    nc.vector.tensor_tensor(
                        ctxu[hp][:, 128 * qb:128 * (qb + 1)], tp[:],
                        ln1x[hp][:, 128 * qb:128 * (qb + 1)], op=ALU.add)
        x2 = ctxu

        # ==== phase 5+6: LN2 + FFN ===========================================
        with tc.tile_pool(name="ffn_sb", bufs=1) as fp:
            # The 1-element seed copies make each weight-load DMA depend on
            # q_sb (ready only after all allgathers are issued) so the bulk
            # weight traffic cannot jump ahead of the kv stores + collective
            # triggers in the DMA queues.
            w_fc_sb = [fp.tile([128, 4 * D], BF16, name=f"wfc{t}")
                       for t in range(ND)]
            for t in range(ND):
                nc.vector.tensor_copy(w_fc_sb[t][0:1, 0:1], q_sb[5][0:1, 0:1])
                nc.sync.dma_start(w_fc_sb[t][:], w_fcT[128 * t:128 * (t + 1), :])
            w_pj_sb = [fp.tile([128, D], BF16, name=f"wpj{m}")
                       for m in range(NFF)]
            for m in range(NFF):
                nc.vector.tensor_copy(w_pj_sb[m][0:1, 0:1], q_sb[5][0:1, 0:1])
                nc.sync.dma_start(w_pj_sb[m][:], w_projT[128 * m:128 * (m + 1), :])
            x2ln = ln1x
            x2lnb = [fp.tile([128, SL], BF16, name=f"x2lnb{t}")
                     for t in range(ND)]
            layernorm_T(x2, ln2w_sb, ln2b_sb, x2ln, x2lnb)
            fps = tc.alloc_tile_pool(name="ffn_ps", bufs=1, space="PSUM")

            # m-major: the proj accumulation for all 6 output tiles advances
            # right after each gelu tile, instead of waiting for all 24
            h_sb = fp.tile([128, NFF * SL], BF16)
            o_ps = [fps.tile([128, SL], F32, tag=f"o{t}", bufs=1,
                             name=f"ops{t}") for t in range(ND)]
            for m in range(NFF):
                ps = fps.tile([128, SL], F32, tag="h", bufs=2)
                for t in range(ND):
                    nc.tensor.matmul(ps[:], w_fc_sb[t][:, 128 * m:128 * (m + 1)],
                                     x2lnb[t][:], start=(t == 0), stop=(t == ND - 1))
                nc.scalar.activation(h_sb[:, SL * m:SL * (m + 1)], ps[:],
                                     AF.Gelu_apprx_tanh,
                                     bias=b_fc_sb[:, m:m + 1])
                for t in range(ND):
                    nc.tensor.matmul(o_ps[t][:],
                                     w_pj_sb[m][:, 128 * t:128 * (t + 1)],
                                     h_sb[:, SL * m:SL * (m + 1)],
                                     start=(m == 0), stop=(m == NFF - 1),
                                     skip_group_check=True)
            for t in range(ND):
                fsum = fp.tile([128, SL], F32, tag="fsum", bufs=2, name=f"fs{t}")
                nc.vector.tensor_scalar(fsum[:], o_ps[t][:], b_proj_sb[:, t:t + 1],
                                        None, op0=ALU.add)
                o = fp.tile([128, SL], F32, tag="out", bufs=2, name=f"o{t}")
                nc.vector.tensor_tensor(o[:], fsum[:], x2ln[t][:], op=ALU.add)
                nc.sync.dma_start(outT[128 * t:128 * (t + 1), :], o[:])
            fps.release()

    nc.compile()
    return nc


# ---- host side --------------------------------------------------------------

def _prep_inputs(cfg, x, ln1_w, ln1_b, w_attn, b_attn, ln2_w, ln2_b,
                 w_fc, b_fc, w_proj, b_proj):
    D, H, NC, SL, ND, NFF = cfg.D, cfg.H, cfg.NC, cfg.SL, cfg.ND, cfg.NFF
    import ml_dtypes
    bf16 = ml_dtypes.bfloat16

    def pp(v, n):  # per-partition layout [128, n]
        return np.ascontiguousarray(v.reshape(n, 128).T.astype(np.float32))

    common = {
        "w_qkT": np.ascontiguousarray(w_attn[:2 * D].T.astype(bf16)),
        "w_vT": np.ascontiguousarray(w_attn[2 * D:].T.astype(bf16)),
        "b_qk": pp(b_attn[:2 * D], 2 * D // 128),
        "b_v": np.ascontiguousarray(b_attn[2 * D:].reshape(1, D).astype(np.float32)),
        "ln1w": pp(ln1_w, ND), "ln1b": pp(ln1_b, ND),
        "ln2w": pp(ln2_w, ND), "ln2b": pp(ln2_b, ND),
        "w_fcT": np.ascontiguousarray(w_fc.T.astype(bf16)),
        "b_fc": pp(b_fc, NFF),
        "w_projT": np.ascontiguousarray(w_proj.T.astype(bf16)),
        "b_proj": pp(b_proj, ND),
    }
    xT = np.ascontiguousarray(x.T.astype(np.float32))
    in_maps = []
    for c in range(NC):
        m = dict(common)
        m["xT"] = np.ascontiguousarray(xT[:, c * SL:(c + 1) * SL])
        in_maps.append(m)
    return in_maps


_CACHE = {}


def kernel(**inputs):
    cfg = Cfg()
    inputs = {k: np.asarray(v) for k, v in inputs.items()}
    in_maps = _prep_inputs(cfg, **inputs)
    if "nc" not in _CACHE:
        _CACHE["nc"] = build(cfg)
    nc = _CACHE["nc"]
    from concourse.bass_utils import run_bass_kernel_spmd
    res = run_bass_kernel_spmd(nc, in_maps, list(range(cfg.NC)))
    outs = [np.asarray(res.results[c]["outT"], dtype=np.float32).T
            for c in range(cfg.NC)]
    return np.ascontiguousarray(np.concatenate(outs, axis=0))
